# revision 16
# baseline (speedup 1.0000x reference)
"""CubicFeatureSampling Trainium2 kernel.

Full-input contract: kernel(ptcloud, cubic_features, neighborhood_size) with
  ptcloud:        [B=4, N=8192, 3]   f32 in [-1, 1]
  cubic_features: [B=4, C=256, S=32, S, S] f32
  neighborhood_size: 1
returns [B, N, K=8, C] f32 (bit-exact vs the jax reference).

Strategy (8 NeuronCores): data-parallel over (batch, half-of-N); each core
handles 4096 points against its batch's feature volume. Host side re-lays
the volume as a zero-padded, channel-last, corner-blocked table
  table[x*S*S + y*S + z] = [f(x+dx, y+dy, z+dz) for k = dx*4+dy*2+dz]
of shape [32768, 8*256] (8KB rows; f == 0 when any coord hits S), so that
  - out-of-bounds corners read exact zeros (no validity-mask multiply), and
  - each point's whole [8, C] output block is ONE 8KB contiguous read,
    already in the reference's corner order.

Device kernel: compute each point's table row index exactly in f32 (all
values < 2^24: floor is made rounding-mode-proof with an is_gt correction),
then issue indirect SWDGE gathers of the HW-verified form "offset [128,1] +
flat dest [128, X]" (partition p <- X contiguous elements starting at table
row off[p]); each gather moves 128 partitions x 8KB, and groups of 8
gathers share one SBUF tile that is stored to the output with a single 8MB
HWDGE DMA, double-buffered via Tile pools. Per core: 32 gathers + 4 stores
saturate HBM (~410 GB/s combined R+W measured).
"""

import numpy as np

import concourse.bass as bass
import concourse.tile as tile
from concourse import mybir
from concourse.bass_utils import run_bass_kernel_spmd

# Problem constants (hardcoded per harness contract).
B = 4
N = 8192
C = 256
S = 32
K = 8
N_CORES = 8
NP = (B * N) // N_CORES   # points per core = 4096

TR = S * S * S            # table rows (32768)
ROW_F32 = K * C           # 2048 floats (8KB) per table row

PTS_PER_PART = NP // 128  # 32 points per partition
# Store-group sizes (sum = 32). The first group is smaller so the first
# store is issued earlier, starting the faster mixed read+write phase
# sooner; slots are sized to the max group.
GROUP_SIZES = [6, 9, 9, 8]

F32 = mybir.dt.float32
I32 = mybir.dt.int32


def _legalize_single_wait(nc):
    """The walrus build here accepts exactly ONE sync wait per instruction
    (setupSyncWait: 'Too many sync wait commands'), but Tile's add_semaphores
    pass emits up to ~3 on DMAs and the tail drain. Hoist all but the last
    wait of each instruction into standalone same-engine InstEventSemaphore
    waits placed immediately before it — semantically identical (the engine
    queue is processed in order)."""
    f = nc.m.functions[0]
    for b in f.blocks:
        out, changed = [], False
        for inst in b.instructions:
            si = inst.sync_info
            if si is not None and si.on_wait and len(si.on_wait) > 1:
                waits = list(si.on_wait)
                for w in waits[:-1]:
                    ev = mybir.InstEventSemaphore(
                        name=nc.get_next_instruction_name(), ins=[], outs=[])
                    ev.engine = inst.engine
                    ev.sync_info = mybir.SyncInfo(on_wait=[w], on_update=[])
                    nc.register_instruction(ev, overwrite=True)
                    out.append(ev)
                inst.sync_info = mybir.SyncInfo(
                    on_wait=[waits[-1]], on_update=list(si.on_update or []))
                changed = True
            out.append(inst)
        if changed:
            b.instructions = out


def build_bass():
    nc = bass.Bass("TRN2")
    pts = nc.declare_dram_parameter("pts", [NP, 3], F32, isOutput=False)
    table = nc.declare_dram_parameter("table", [TR, ROW_F32], F32,
                                      isOutput=False)
    out = nc.declare_dram_parameter("out", [NP * K, C], F32, isOutput=True)

    # Partition p owns points p*32..p*32+31; output rows for point
    # p*32+q land at (p*32+q)*8 + k, i.e. partition stride 256 rows.
    outv = out[:].rearrange("(p u) d -> p (u d)", p=128)  # [128, 256*C]

    with tile.TileContext(nc) as tc:
        with (
            tc.tile_pool(name="gather", bufs=2) as gpool,
            tc.tile_pool(name="idx", bufs=1) as ipool,
        ):
            pt_all = ipool.tile([128, PTS_PER_PART * 3], F32, tag="ptall")
            nc.sync.dma_start(
                out=pt_all[:],
                in_=pts[:].rearrange("(p w) t -> p (w t)", p=128))

            # ---- t = pt*16 + 16  (pt*16 is exact; one rounding on +16,
            # identical to the reference's f32 computation)
            t = ipool.tile([128, PTS_PER_PART * 3], F32, tag="t")
            nc.scalar.activation(
                out=t[:], in_=pt_all[:],
                func=mybir.ActivationFunctionType.Copy,
                bias=float(S) / 2.0, scale=float(S) / 2.0)

            # ---- exact floor(t), robust to the f32->i32 rounding mode:
            # gi = int(t); gf = float(gi); gf -= (gf > t)
            gi = ipool.tile([128, PTS_PER_PART * 3], I32, tag="gi")
            nc.vector.tensor_copy(gi[:], t[:])
            gf = ipool.tile([128, PTS_PER_PART * 3], F32, tag="gf")
            nc.vector.tensor_copy(gf[:], gi[:])
            corr = ipool.tile([128, PTS_PER_PART * 3], F32, tag="corr")
            nc.vector.tensor_tensor(
                out=corr[:], in0=gf[:], in1=t[:], op=mybir.AluOpType.is_gt)
            nc.vector.tensor_tensor(
                out=gf[:], in0=gf[:], in1=corr[:],
                op=mybir.AluOpType.subtract)

            # ---- row = gx*S*S + gy*S + gz   (exact in f32)
            g3 = gf[:].rearrange("p (w t) -> p w t", t=3)
            t1 = ipool.tile([128, PTS_PER_PART], F32, tag="t1")
            nc.vector.scalar_tensor_tensor(
                out=t1[:], in0=g3[:, :, 1], scalar=float(S),
                in1=g3[:, :, 2],
                op0=mybir.AluOpType.mult, op1=mybir.AluOpType.add)
            base = ipool.tile([128, PTS_PER_PART], F32, tag="base")
            nc.vector.scalar_tensor_tensor(
                out=base[:], in0=g3[:, :, 0], scalar=float(S * S),
                in1=t1[:],
                op0=mybir.AluOpType.mult, op1=mybir.AluOpType.add)

            lin = ipool.tile([128, PTS_PER_PART], I32, tag="lin")
            nc.vector.tensor_copy(lin[:], base[:])

            # ---- gather + store, double buffered by group
            off = 0
            for sz in GROUP_SIZES:
                gt = gpool.tile([128, sz * ROW_F32], F32, tag="gt")
                for jj in range(sz):
                    j = off + jj
                    nc.gpsimd.indirect_dma_start(
                        out=gt[:, jj * ROW_F32:(jj + 1) * ROW_F32],
                        out_offset=None,
                        in_=table[:],
                        in_offset=bass.IndirectOffsetOnAxis(
                            ap=lin[:, j:j + 1], axis=0),
                    )
                nc.sync.dma_start(
                    out=outv[:, off * ROW_F32:(off + sz) * ROW_F32],
                    in_=gt[:, :sz * ROW_F32])
                off += sz

    _legalize_single_wait(nc)
    return nc


def _build_table(cubic_b):
    """[C,S,S,S] -> corner-blocked table [S^3, 8*C] f32.
    Row (x*S + y)*S + z holds the 8 corner feature vectors of cell
    (x, y, z) in order k = dx*4 + dy*2 + dz, zeros where a coord == S."""
    pad = np.zeros((S + 1, S + 1, S + 1, C), dtype=np.float32)
    pad[:S, :S, :S] = np.transpose(cubic_b, (1, 2, 3, 0))
    t = np.empty((S, S, S, K, C), dtype=np.float32)
    for k in range(K):
        dx, dy, dz = (k >> 2) & 1, (k >> 1) & 1, k & 1
        t[:, :, :, k] = pad[dx:S + dx, dy:S + dy, dz:S + dz]
    return t.reshape(TR, ROW_F32)


def _shard_inputs(ptcloud, cubic_features):
    """Build the 8 per-core input maps (host-side data-parallel sharding)."""
    ptcloud = np.ascontiguousarray(ptcloud, dtype=np.float32)
    cubic_features = np.asarray(cubic_features, dtype=np.float32)
    half = N // 2
    in_maps = []
    for b in range(B):
        tb = _build_table(cubic_features[b])
        for h in range(2):
            in_maps.append({
                "pts": np.ascontiguousarray(
                    ptcloud[b, h * half:(h + 1) * half]),
                "table": tb,
            })
    return in_maps


def _gather_output(results):
    half = N // 2
    out = np.empty((B, N, K, C), dtype=np.float32)
    for ci, r in enumerate(results):
        b, h = divmod(ci, 2)
        out[b, h * half:(h + 1) * half] = r["out"].reshape(half, K, C)
    return out


def run(ptcloud, cubic_features, trace=False):
    """Shard, run on 8 cores, unshard. Returns (output, BassKernelResults)."""
    in_maps = _shard_inputs(ptcloud, cubic_features)
    nc = build_bass()
    res = run_bass_kernel_spmd(
        nc, in_maps, core_ids=list(range(N_CORES)), trace=trace)
    return _gather_output(res.results), res


def kernel(ptcloud, cubic_features, neighborhood_size):
    assert int(neighborhood_size) == 1
    out, _ = run(ptcloud, cubic_features)
    return out
